# revision 34
# baseline (speedup 1.0000x reference)
"""Causal self-attention (B=4, S=2048, H=2048, 16 heads) on 8 Trainium2 NeuronCores.

Sharding: DP4 over batch x TP2 over heads. Core c handles batch c//2 and head
half c%2 (8 heads of 128 dims). fp16 matmul operands throughout (PSUM fp32).

v3 (vs v2):
  - Weights are repacked on the host into the exact SBUF tile layout, so the
    q/k weight DMAs are 4KB-contiguous per partition line (v2 used 256B
    descriptors, 8.4MB of them, which capped startup DMA at ~220GB/s).
  - DMA is split across both hardware DGE queues: x strips on the Sync
    queue, weights on the Scalar queue, so the first matmul starts ~4us in.
  - Softmax denominators: exp pair tiles are pre-summed in groups of 4 and
    contracted against an all-ones [128,128] stationary tile, so the den
    arrives in PSUM already broadcast across all 128 partitions.  1/den via
    a single reciprocal_approx_fast custom-DVE op reading PSUM directly.
    This kills the v2 DRAM den bounce (3 DMA hops + a 3.3us blocking DVE
    reciprocal per head that stalled PE ~2us/head) and the per-qb broadcast
    matmuls.
  - Output projection unchanged (it already ran at full PE rate).
"""

import math
import sys

if "/opt/trn_rl_repo" not in sys.path:
    sys.path.insert(0, "/opt/trn_rl_repo")

import numpy as np

B, S, HID = 4, 2048, 2048
HEADS, D = 16, 128
HH = HEADS // 2          # heads per core
HHID = HH * D            # 1024, per-core head-span of hidden
KT = HID // 128          # 16 contraction tiles of 128
NB = S // 512            # 4 free-dim blocks of 512
N_CORES = 8
NCHUNK = 4               # ctx-exchange chunks (2 heads each)

_CACHED = {}


def _build_program():
    import concourse.tile as tile
    import concourse.mybir as mybir
    from concourse import bacc
    from concourse._compat import get_trn_type

    F32 = mybir.dt.float32
    F16 = mybir.dt.float16
    Exp = mybir.ActivationFunctionType.Exp
    Identity = mybir.ActivationFunctionType.Identity
    Copy = mybir.ActivationFunctionType.Copy

    nc = bacc.Bacc(
        get_trn_type() or "TRN2",
        target_bir_lowering=False,
        debug=False,
        enable_asserts=False,
        num_devices=N_CORES,
    )

    def din(name, shape, dt=F16):
        return nc.dram_tensor(name, shape, dt, kind="ExternalInput").ap()

    xT = din("xT", [HID, S])            # x[b].T, fp16
    # host-prepacked: [p, (h*KT+g)*128 + c] = WqT[g*128+p, h*128+c]
    wqT = din("wqT", [128, HH * KT * 128])
    wkT = din("wkT", [128, HH * KT * 128])
    # host-prepacked: [p, (g2*KT+k)*512 + c] = WvT[k*128+p, g2*512+c]
    wvT = din("wvT", [128, 2 * KT * 512])
    woT = din("woT", [HID, HHID])       # Wo.T columns for this core's o-half
    bq = din("bq", [128, HH], F32)      # bq[h*128+p] at [p, h]
    bk = din("bk", [128, HH], F32)
    bo = din("bo", [1, HHID], F16)      # bo_eff slice for this core's o-half
    masks = din("masks", [4, 128, 512])
    out = nc.dram_tensor("out", [S, HHID], F16, kind="ExternalOutput").ap()

    inv_sqrt_d = float(1.0 / math.sqrt(D))

    with tile.TileContext(nc) as tc, \
         nc.allow_low_precision(reason="fp16 operand pipeline"):
        with tc.tile_pool(name="const", bufs=1) as constp, \
             tc.tile_pool(name="dram", bufs=1, space="DRAM") as dramp:
            ctx_send = [dramp.tile([256, S], F16, tag=f"ctxs{c}",
                                   name=f"ctxs{c}") for c in range(NCHUNK)]
            ctx_recv = [dramp.tile([512, S], F16, tag=f"ctxr{c}",
                                   name=f"ctxr{c}") for c in range(NCHUNK)]

            # constants: tiles declared here, DMAs issued after the hot
            # startup weight loads (scalar-queue triggers run in program
            # order, and wq0/wk0 gate the first matmul)
            ones_row = constp.tile([1, 128], F16, tag="ones_row")
            nc.vector.memset(ones_row, 1.0)
            ones_sq = constp.tile([128, 128], F16, tag="ones_sq")
            nc.vector.memset(ones_sq, 1.0)
            mask_t = [constp.tile([128, 512], F16, tag=f"mask{r}",
                                  name=f"mask{r}") for r in range(4)]
            bq_sb = constp.tile([128, HH], F32, tag="bq_sb")
            bk_sb = constp.tile([128, HH], F32, tag="bk_sb")
            bo_sb = constp.tile([1, HHID], F16, tag="bo_sb")

            def load_biases():
                # sync queue (fast(0)'s Scalar activations consume them, so
                # they must not trail those in the Scalar stream)
                nc.sync.dma_start(out=bq_sb, in_=bq)
                nc.sync.dma_start(out=bk_sb, in_=bk)
                nc.sync.dma_start(out=bo_sb, in_=bo)

            def load_masks():
                for r in range(4):
                    nc.sync.dma_start(out=mask_t[r], in_=masks[r])

            with tc.tile_pool(name="xk", bufs=KT) as xp, \
                 tc.tile_pool(name="p1w", bufs=4) as wp, \
                 tc.tile_pool(name="p1wv", bufs=1) as wvp, \
                 tc.tile_pool(name="p1qk", bufs=6) as qkp, \
                 tc.tile_pool(name="p1v", bufs=2 * KT) as vp, \
                 tc.tile_pool(name="p2et", bufs=6) as etp, \
                 tc.tile_pool(name="p2es", bufs=4) as esp, \
                 tc.tile_pool(name="p2cu", bufs=6) as cup, \
                 tc.tile_pool(name="p2r", bufs=2) as rpp, \
                 tc.tile_pool(name="p2c", bufs=2) as cp, \
                 tc.tile_pool(name="ps1", bufs=2, space="PSUM") as pp, \
                 tc.tile_pool(name="ps2s", bufs=2, space="PSUM") as pps, \
                 tc.tile_pool(name="ps2c", bufs=1, space="PSUM") as ppc, \
                 tc.tile_pool(name="ps2d", bufs=1, space="PSUM") as ppd:

                wtiles = {}
                qk_sb = {}
                wv_sb = {}
                v4 = {0: [None] * KT, 1: [None] * KT}
                xk = [None] * KT

                def qk_w_alloc(h):
                    for pname in ("q", "k"):
                        wtiles[(pname, h)] = wp.tile(
                            [128, KT, 128], F16, tag="w", name=f"w{pname}{h}")

                def qk_w_split(h, pname, g0, g1, eng):
                    # prepacked: per-head slab is contiguous [128, KT*128]
                    wT = wqT if pname == "q" else wkT
                    eng.dma_start(
                        out=wtiles[(pname, h)][:, g0:g1, :],
                        in_=wT[:, (h * KT + g0) * 128:
                               (h * KT + g1) * 128].rearrange(
                            "p (g c) -> p g c", c=128))

                def qk_w(h, nsplit=1, eng=None):
                    eng = eng or nc.sync
                    qk_w_alloc(h)
                    gs = KT // nsplit
                    for pname in ("q", "k"):
                        for sp_ in range(nsplit):
                            qk_w_split(h, pname, sp_ * gs, (sp_ + 1) * gs,
                                       eng)

                def load_x(ks, eng=None):
                    eng = eng or nc.sync
                    for k in ks:
                        t = xp.tile([128, S], F16, tag="xk", name=f"xk{k}")
                        eng.dma_start(out=t, in_=xT[k * 128:(k + 1) * 128, :])
                        xk[k] = t

                def qk_mm(h, ns=None):
                    for pname, bias_sb in (("q", bq_sb), ("k", bk_sb)):
                        if ns is None or 0 in ns:
                            w = wtiles[(pname, h)]
                            dst = qkp.tile([128, S], F16, tag="qk",
                                           name=f"{pname}h{h}")
                            qk_sb[(pname, h)] = dst
                        else:
                            w = wtiles[(pname, h)]
                            dst = qk_sb[(pname, h)]
                        for n in (range(NB) if ns is None else ns):
                            ps = pp.tile([128, 512], F32, tag="ps1",
                                         name=f"ps{pname}{h}_{n}")
                            for k in range(KT):
                                nc.tensor.matmul(
                                    ps, w[:, k, :],
                                    xk[k][:, n * 512:(n + 1) * 512],
                                    start=(k == 0), stop=(k == KT - 1))
                            nc.scalar.activation(
                                out=dst[:, n * 512:(n + 1) * 512], in_=ps,
                                func=Identity, bias=bias_sb[:, h:h + 1])

                def qk_mm_fast(h):
                    # startup variant: 4 concurrent psum groups per proj,
                    # k-outer so matmuls chase the arriving x strips
                    for pname, bias_sb in (("q", bq_sb), ("k", bk_sb)):
                        w = wtiles[(pname, h)]
                        dst = qkp.tile([128, S], F16, tag="qk",
                                       name=f"{pname}h{h}")
                        qk_sb[(pname, h)] = dst
                        ps01 = [pp.tile([128, 512], F32, tag="ps1",
                                        name=f"f{pname}{h}_{n}")
                                for n in range(2)]
                        big = pps.tile([128, 1024], F32, tag="sps",
                                       name=f"f{pname}{h}_23")
                        pss = ps01 + [big[:, 0:512], big[:, 512:1024]]
                        for k in range(KT):
                            for n in range(NB):
                                nc.tensor.matmul(
                                    pss[n], w[:, k, :],
                                    xk[k][:, n * 512:(n + 1) * 512],
                                    start=(k == 0), stop=(k == KT - 1))
                        for n in range(NB):
                            nc.scalar.activation(
                                out=dst[:, n * 512:(n + 1) * 512],
                                in_=pss[n], func=Identity,
                                bias=bias_sb[:, h:h + 1])

                def qk_mm_fast2(h):
                    # startup variant: q AND k projections interleaved per
                    # x strip, 8 concurrent psum groups (borrows the
                    # attention pools' banks, idle during startup), so the
                    # DMA-paced strip window does both projections' work
                    dsts = {}
                    for pname in ("q", "k"):
                        dst = qkp.tile([128, S], F16, tag="qk",
                                       name=f"{pname}h{h}")
                        qk_sb[(pname, h)] = dst
                        dsts[pname] = dst
                    big_q = pps.tile([128, 1024], F32, tag="sps",
                                     name=f"f2q{h}")
                    big_k = pps.tile([128, 1024], F32, tag="sps",
                                     name=f"f2k{h}")
                    q_pss = [pp.tile([128, 512], F32, tag="ps1",
                                     name=f"f2q{h}_{n}") for n in range(2)] \
                        + [big_q[:, 0:512], big_q[:, 512:1024]]
                    k_pss = [ppc.tile([128, 512], F32, tag="ctxps",
                                      name=f"f2k{h}_0"),
                             ppd.tile([128, 512], F32, tag="denps",
                                      name=f"f2k{h}_1"),
                             big_k[:, 0:512], big_k[:, 512:1024]]
                    wq_, wk_ = wtiles[("q", h)], wtiles[("k", h)]
                    for k in range(KT):
                        for n in range(NB):
                            nc.tensor.matmul(
                                q_pss[n], wq_[:, k, :],
                                xk[k][:, n * 512:(n + 1) * 512],
                                start=(k == 0), stop=(k == KT - 1))
                            nc.tensor.matmul(
                                k_pss[n], wk_[:, k, :],
                                xk[k][:, n * 512:(n + 1) * 512],
                                start=(k == 0), stop=(k == KT - 1))
                    for pname, pss, bias_sb in (("q", q_pss, bq_sb),
                                                ("k", k_pss, bk_sb)):
                        for n in range(NB):
                            nc.scalar.activation(
                                out=dsts[pname][:, n * 512:(n + 1) * 512],
                                in_=pss[n], func=Identity,
                                bias=bias_sb[:, h:h + 1])

                def v_w(g, eng=None):
                    eng = eng or nc.sync
                    w = wvp.tile([128, KT, 512], F16, tag="wv",
                                 name=f"wv{g}")
                    eng.dma_start(
                        out=w,
                        in_=wvT[:, g * KT * 512:(g + 1) * KT * 512].rearrange(
                            "p (k c) -> p k c", c=512))
                    wv_sb[g] = w

                def v_mm(g):
                    w = wv_sb.pop(g)
                    for m in range(KT):
                        ps = pp.tile([128, 512], F32, tag="ps1",
                                     name=f"psv{g}_{m}")
                        for k in range(KT):
                            nc.tensor.matmul(
                                ps, xk[k][:, m * 128:(m + 1) * 128],
                                w[:, k, :],
                                start=(k == 0), stop=(k == KT - 1))
                        vsb = vp.tile([128, 512], F16, tag="v4",
                                      name=f"v{g}_{m}")
                        nc.vector.tensor_copy(out=vsb, in_=ps)
                        v4[g][m] = vsb

                def attention(h, fillers=None):
                    g, sub = h // 4, h % 4
                    qh = qk_sb.pop(("q", h))
                    kh = qk_sb.pop(("k", h))
                    ct16 = cp.tile([128, S], F16, tag="ctxh", name=f"ctxh{h}")
                    for qb in range(NB):
                        if fillers and fillers.get(qb):
                            fillers[qb]()
                        kept = 4 * qb + 4
                        q0 = qb * 512
                        ctx_ps = ppc.tile([128, 512], F32, tag="ctxps",
                                          name=f"cps{h}_{qb}")
                        den_ps = ppd.tile([128, 512], F32, tag="denps",
                                          name=f"dps{h}_{qb}")
                        for g4 in range(0, kept, 4):
                            # per-kt trim: diagonal tile r only touches
                            # q-cols >= 128r (cols below are fully masked);
                            # off/width per kt in this group of 4
                            ow = []
                            for kt in range(g4, g4 + 4):
                                r = kt - 4 * qb
                                o = 128 * r if r > 0 else 0
                                ow.append((kt, o, 512 - o))
                            ets = {}
                            for half in range(2):
                                (ktA, oA, wA), (ktB, oB, wB) = \
                                    ow[2 * half:2 * half + 2]
                                sps = pps.tile([128, 1024], F32, tag="sps",
                                               name=f"sps{h}_{qb}_{ktA}")
                                nc.tensor.matmul(
                                    sps[:, 0:wA],
                                    kh[:, ktA * 128:(ktA + 1) * 128],
                                    qh[:, q0 + oA:q0 + 512],
                                    start=True, stop=True)
                                nc.tensor.matmul(
                                    sps[:, wA:wA + wB],
                                    kh[:, ktB * 128:(ktB + 1) * 128],
                                    qh[:, q0 + oB:q0 + 512],
                                    start=True, stop=True)
                                et = etp.tile([128, 1024], F16, tag="et",
                                              name=f"et{h}_{qb}_{ktA}")
                                nc.scalar.activation(
                                    out=et[:, 0:wA + wB],
                                    in_=sps[:, 0:wA + wB],
                                    func=Exp, scale=inv_sqrt_d)
                                for kt, o, w, sl in ((ktA, oA, wA,
                                                      slice(0, wA)),
                                                     (ktB, oB, wB,
                                                      slice(wA, wA + wB))):
                                    e = et[:, sl]
                                    if kt - 4 * qb >= 0:
                                        nc.vector.tensor_mul(
                                            e, e, mask_t[0][:, 0:w])
                                    ets[kt] = e
                                    nc.tensor.matmul(
                                        ctx_ps[:, o:512],
                                        v4[g][kt][:, sub * 128:
                                                  (sub + 1) * 128],
                                        e,
                                        start=(kt == 0),
                                        stop=(kt == kept - 1))
                            es4 = esp.tile([128, 512], F16, tag="es",
                                           name=f"es4_{h}_{qb}_{g4}")
                            (ktA, oA, wA), (ktB, oB, wB), (ktC, oC, wC), \
                                (ktD, oD, wD) = ow
                            if oB == 0:
                                # off-diagonal group: all widths 512
                                ea = esp.tile([128, 512], F16, tag="es",
                                              name=f"ea{h}_{qb}_{g4}")
                                nc.vector.tensor_add(ea, ets[ktA], ets[ktB])
                                nc.vector.tensor_add(es4, ets[ktC], ets[ktD])
                                nc.vector.tensor_add(es4, es4, ea)
                            else:
                                # diagonal group: widths 512/384/256/128 at
                                # offsets 0/128/256/384
                                nc.vector.tensor_copy(
                                    out=es4[:, 0:128], in_=ets[ktA][:, 0:128])
                                nc.vector.tensor_add(
                                    es4[:, 128:512], ets[ktA][:, 128:512],
                                    ets[ktB])
                                nc.vector.tensor_add(
                                    es4[:, 256:512], es4[:, 256:512],
                                    ets[ktC])
                                nc.vector.tensor_add(
                                    es4[:, 384:512], es4[:, 384:512],
                                    ets[ktD])
                            # pre-sum pairs of 4-groups into 8-groups where
                            # possible: halves the den matmul count
                            if g4 % 8 == 0 and g4 + 4 < kept:
                                es_hold = es4
                            elif g4 % 8 == 4:
                                nc.vector.tensor_add(es4, es4, es_hold)
                                nc.tensor.matmul(den_ps, ones_sq, es4,
                                                 start=(g4 == 4),
                                                 stop=(g4 >= kept - 4))
                            else:
                                # lone trailing / only 4-group
                                nc.tensor.matmul(den_ps, ones_sq, es4,
                                                 start=(g4 == 0),
                                                 stop=(g4 >= kept - 4))
                        rp = rpp.tile([128, 512], F32, tag="rp",
                                      name=f"rp{h}_{qb}")
                        nc.vector.reciprocal_approx_fast(out=rp, in_=den_ps)
                        cu = cup.tile([128, 512], F32, tag="cu",
                                      name=f"cu{h}_{qb}")
                        nc.scalar.activation(out=cu, in_=ctx_ps, func=Copy)
                        nc.vector.tensor_mul(
                            ct16[:, qb * 512:(qb + 1) * 512], cu, rp)
                    nc.sync.dma_start(
                        out=ctx_send[h // 2][(h % 2) * 128:
                                             (h % 2) * 128 + 128, :],
                        in_=ct16)
                    if h % 2 == 1:
                        nc.gpsimd.collective_compute(
                            "AllGather",
                            mybir.AluOpType.bypass,
                            replica_groups=[[0, 1], [2, 3], [4, 5], [6, 7]],
                            ins=[ctx_send[h // 2].opt()],
                            outs=[ctx_recv[h // 2].opt()],
                        )

                # interleaved emission: weights-first start, projections
                # feeding attention per head
                # startup: head-0/1 weights + odd x strips ride the (idle)
                # Scalar hwdge queue, even x strips the Sync queue (a single
                # DGE queue tops out ~250GB/s).  Later weights go on the
                # Sync queue one head ahead of use — Sync's stream runs far
                # ahead, while Scalar's is backlogged by exp work.
                # emit strictly in need-order: DMA-completion semaphores are
                # shared round-robin over all DMAs, so each transfer's
                # readiness chains on the 9-back emission — a late-emitted
                # early-needed tile would inherit a mid-queue wait
                qk_w_alloc(0)
                qk_w_alloc(1)
                # first strip split across both queues: halves its latency,
                # and xk0 gates the very first matmul
                xk[0] = xp.tile([128, S], F16, tag="xk", name="xk0")
                nc.sync.dma_start(out=xk[0][0:64, :], in_=xT[0:64, :])
                nc.scalar.dma_start(out=xk[0][64:128, :], in_=xT[64:128, :])
                qk_w_split(0, "q", 0, 4, nc.scalar)
                qk_w_split(0, "k", 0, 4, nc.scalar)
                load_x([1], nc.scalar)
                load_x([2])
                qk_w_split(0, "q", 4, 8, nc.scalar)
                qk_w_split(0, "k", 4, 8, nc.scalar)
                load_x([3], nc.scalar)
                load_x([4])
                load_biases()
                qk_w_split(0, "q", 8, 12, nc.scalar)
                qk_w_split(0, "k", 8, 12, nc.scalar)
                load_x([5], nc.scalar)
                load_x([6])
                qk_w_split(0, "q", 12, 16, nc.scalar)
                qk_w_split(0, "k", 12, 16, nc.scalar)
                load_x([7], nc.scalar)
                load_x([8])
                load_x([9], nc.scalar)
                load_x([10])
                load_x([11], nc.scalar)
                load_x([12])
                load_x([13], nc.scalar)
                load_x([14])
                load_x([15], nc.scalar)
                qk_w_split(1, "q", 0, 8, nc.scalar)
                qk_w_split(1, "q", 8, 16, nc.scalar)
                qk_w_split(1, "k", 0, 8, nc.scalar)
                qk_w_split(1, "k", 8, 16, nc.scalar)
                v_w(0, eng=nc.scalar)
                load_masks()
                qk_w(2)
                qk_w(3)
                qk_mm_fast2(0)
                qk_mm_fast(1)
                v_mm(0)
                qk_mm(2)
                attention(0)
                qk_w(4)
                qk_mm(3)
                attention(1)
                v_w(1)
                qk_w(5)
                v_mm(1)
                qk_mm(4)
                attention(2)
                qk_w(6)
                qk_mm(5)
                attention(3)
                qk_w(7)
                qk_mm(6)
                attention(4)
                qk_mm(7, ns=[0, 1])
                attention(5)
                # q7/k7 n-blocks 2,3 as PE filler inside attention(6):
                # pure attention is Scalar(exp)-paced, these chains keep PE fed
                attention(6, fillers={
                    0: lambda: qk_mm(7, ns=[2]),
                    1: lambda: qk_mm(7, ns=[3]),
                })
                attention(7)

            # -------- phase 4: chunk-major output projection --------
            with tc.tile_pool(name="p4wo", bufs=16) as wop, \
                 tc.tile_pool(name="p4ct", bufs=16) as ctp, \
                 tc.tile_pool(name="p4acc", bufs=16) as accp, \
                 tc.tile_pool(name="p4o", bufs=3) as op_, \
                 tc.tile_pool(name="p4b", bufs=1) as bp4, \
                 tc.tile_pool(name="ps4", bufs=4, space="PSUM") as pp4:
                # broadcast bo across partitions via ones outer product
                bo_bc = bp4.tile([128, HHID], F32, tag="bo_bc")
                for n in range(HHID // 512):
                    bps = pp4.tile([128, 512], F32, tag="ps4", name=f"bps{n}")
                    nc.tensor.matmul(bps, ones_row,
                                     bo_sb[:, n * 512:(n + 1) * 512],
                                     start=True, stop=True)
                    nc.vector.tensor_copy(out=bo_bc[:, n * 512:(n + 1) * 512],
                                          in_=bps)
                acc = [accp.tile([128, HHID], F32, tag="acc", name=f"acc{m}")
                       for m in range(S // 128)]
                for c in range(NCHUNK):
                    cts = []
                    for off, gk in ((0, 2 * c), (128, 2 * c + 1),
                                    (256, 8 + 2 * c), (384, 8 + 2 * c + 1)):
                        t = ctp.tile([128, S], F16, tag="ct", name=f"ct{gk}")
                        nc.sync.dma_start(out=t,
                                          in_=ctx_recv[c][off:off + 128, :])
                        w = wop.tile([128, HHID], F16, tag="wo",
                                     name=f"wo{gk}")
                        nc.sync.dma_start(
                            out=w, in_=woT[gk * 128:(gk + 1) * 128, :])
                        cts.append((t, w))
                    for m in range(S // 128):
                        pss = [pp4.tile([128, 512], F32, tag="ps4",
                                        name=f"ps4_{c}_{m}_{n}")
                               for n in range(HHID // 512)]
                        for ki, (t, w) in enumerate(cts):
                            for n in range(HHID // 512):
                                nc.tensor.matmul(
                                    pss[n], t[:, m * 128:(m + 1) * 128],
                                    w[:, n * 512:(n + 1) * 512],
                                    start=(ki == 0), stop=(ki == 3))
                        if c == 0:
                            for n in range(HHID // 512):
                                sl = slice(n * 512, (n + 1) * 512)
                                nc.vector.tensor_add(acc[m][:, sl], pss[n],
                                                     bo_bc[:, sl])
                        elif c < NCHUNK - 1:
                            for n in range(HHID // 512):
                                sl = slice(n * 512, (n + 1) * 512)
                                nc.vector.tensor_add(acc[m][:, sl],
                                                     acc[m][:, sl], pss[n])
                        else:
                            ot = op_.tile([128, HHID], F16, tag="osb",
                                          name=f"osb{m}")
                            for n in range(HHID // 512):
                                sl = slice(n * 512, (n + 1) * 512)
                                nc.vector.tensor_add(ot[:, sl],
                                                     acc[m][:, sl], pss[n])
                            nc.sync.dma_start(
                                out=out[m * 128:(m + 1) * 128, :], in_=ot)

    nc.compile()
    return nc


def _get_nc():
    if "nc" not in _CACHED:
        _CACHED["nc"] = _build_program()
    return _CACHED["nc"]


def _make_masks():
    i = np.arange(128)[:, None]
    j = np.arange(512)[None, :]
    return np.stack(
        [((j - i) >= 128 * r).astype(np.float16) for r in range(4)], axis=0)


def _pack_qk(WT_core):
    # [2048, 1024] -> [128, HH*KT*128] with
    # [p, (h*KT+g)*128 + c] = WT[g*128+p, h*128+c]
    w = WT_core.reshape(KT, 128, HH, 128).transpose(1, 2, 0, 3)
    return np.ascontiguousarray(w.reshape(128, HH * KT * 128))


def _pack_v(WT_core):
    # [2048, 1024] -> [128, 2*KT*512] with
    # [p, (g2*KT+k)*512 + c] = WT[k*128+p, g2*512+c]
    w = WT_core.reshape(KT, 128, 2, 512).transpose(1, 2, 0, 3)
    return np.ascontiguousarray(w.reshape(128, 2 * KT * 512))


def _make_in_maps(inputs):
    x = np.ascontiguousarray(np.asarray(inputs["x"], dtype=np.float32))
    Wq = np.asarray(inputs["Wq"], dtype=np.float32)
    Wk = np.asarray(inputs["Wk"], dtype=np.float32)
    Wv = np.asarray(inputs["Wv"], dtype=np.float32)
    Wo = np.asarray(inputs["Wo"], dtype=np.float32)
    bq = np.asarray(inputs["bq"], dtype=np.float32)
    bk = np.asarray(inputs["bk"], dtype=np.float32)
    bv = np.asarray(inputs["bv"], dtype=np.float32)
    bo = np.asarray(inputs["bo"], dtype=np.float32)

    bo_eff = bo + Wo @ bv
    masks = _make_masks()
    WqT = np.ascontiguousarray(Wq.T)
    WkT = np.ascontiguousarray(Wk.T)
    WvT = np.ascontiguousarray(Wv.T)
    WoT = np.ascontiguousarray(Wo.T)

    in_maps = []
    for c in range(N_CORES):
        b, hf = c // 2, c % 2
        sl = slice(hf * HHID, (hf + 1) * HHID)
        in_maps.append({
            "xT": np.ascontiguousarray(x[b].T).astype(np.float16),
            "wqT": _pack_qk(WqT[:, sl]).astype(np.float16),
            "wkT": _pack_qk(WkT[:, sl]).astype(np.float16),
            "wvT": _pack_v(WvT[:, sl]).astype(np.float16),
            "woT": np.ascontiguousarray(WoT[:, sl]).astype(np.float16),
            "bq": np.ascontiguousarray(bq[sl].reshape(HH, 128).T),
            "bk": np.ascontiguousarray(bk[sl].reshape(HH, 128).T),
            "bo": bo_eff[sl].reshape(1, HHID).astype(np.float16),
            "masks": masks,
        })
    return in_maps


def kernel(**inputs):
    from concourse.bass_utils import run_bass_kernel_spmd

    in_maps = _make_in_maps(inputs)
    nc = _get_nc()
    res = run_bass_kernel_spmd(nc, in_maps, list(range(N_CORES)))

    out = np.empty((B, S, HID), dtype=np.float32)
    for c in range(N_CORES):
        b, hf = c // 2, c % 2
        out[b, :, hf * HHID:(hf + 1) * HHID] = res.results[c]["out"]
    return out
